# revision 48
# baseline (speedup 1.0000x reference)
"""8-bit ripple-carry adder on 8 TRN2 NeuronCores.

Full inputs A[N,8], B[N,8] (MSB-first bits in {0,1} as f32), Cin[N,1].
Returns (out[N,8], carry[N,1]) matching the reference.

Strategy: pure data-parallel over the batch dim (N/8 rows per core).
Per core, rows are tiled [128 partitions x F rows]. Per row the carry
recurrence is c' = (a + b + c >= 2); with t = a + b laid out in 10-wide
groups [b7..b0, slot, slot] (slot = 2*Cin, which forces the next state to
Cin and so resets the chain between rows), a single reversed DVE
tensor_tensor_scan  state = (t + state) >= 2  computes every carry of
every row. Sum bits are (t == 1) XOR carry_in.

bf16 is used for all DVE elementwise ops (values stay in {0,1,2} -
exact), keeping them in the DVE's 2x/4x perf modes; f32<->bf16 converts
run on the Scalar engine, which has its own SBUF ports. GPSIMD is
deliberately unused (it shares SBUF ports with the DVE; concurrent
GPSIMD ops slow DVE tensor ops ~3x, measured). Deep tile-pool buffering
on the input side keeps the DMA stream running ahead of compute.
"""

import sys

import numpy as np

if "/opt/trn_rl_repo" not in sys.path:
    sys.path.insert(0, "/opt/trn_rl_repo")

N_BITS = 8
P = 128
N_CORES = 8


def build_adder_nc(R: int, F: int):
    """Per-core Bass program for an R-row shard, F rows/partition/tile."""
    import concourse.bacc as bacc
    import concourse.mybir as mybir
    from concourse.mybir import AluOpType
    from concourse.tile import TileContext, add_dep_helper

    f32 = mybir.dt.float32
    bf16 = mybir.dt.bfloat16
    Copy = mybir.ActivationFunctionType.Copy
    G = 10  # group width: 8 bits MSB-first + 2 reset slots

    # uniform tiles of F rows/partition (non-uniform head/tail tiles were
    # measured slower - the schedule is sensitive to the steady rhythm)
    rpp = R // P
    assert R % P == 0 and rpp % F == 0
    fseq = [F] * (rpp // F)
    T = len(fseq)
    row0s = []
    r0 = 0
    for fi in fseq:
        row0s.append(r0)
        r0 += P * fi

    nc = bacc.Bacc("TRN2", target_bir_lowering=False, debug=False)

    A = nc.declare_dram_parameter("A", [R, N_BITS], f32, isOutput=False)
    B = nc.declare_dram_parameter("B", [R, N_BITS], f32, isOutput=False)
    CIN = nc.declare_dram_parameter("Cin", [R, 1], f32, isOutput=False)
    OUT = nc.declare_dram_parameter("out", [R, N_BITS], f32, isOutput=True)
    COUT = nc.declare_dram_parameter("cout", [R, 1], f32, isOutput=True)

    A_f = A[:].flatten()
    B_f = B[:].flatten()
    C_f = CIN[:].flatten()
    O_f = OUT[:].flatten()
    K_f = COUT[:].flatten()

    def bits_ap(flat, i):
        r0, fi = row0s[i], fseq[i]
        return flat[r0 * N_BITS : (r0 + P * fi) * N_BITS].rearrange(
            "(p x) -> p x", p=P
        )

    def col_ap(flat, i):
        r0, fi = row0s[i], fseq[i]
        return flat[r0 : r0 + P * fi].rearrange("(p x) -> p x", p=P)

    with TileContext(nc) as tc:
        with (
            tc.tile_pool(name="const", bufs=1) as const_pool,
            tc.tile_pool(name="io", bufs=4) as io_pool,
            tc.tile_pool(name="cin", bufs=6) as cin_pool,
            tc.tile_pool(name="work", bufs=3) as work_pool,
            tc.tile_pool(name="outp", bufs=3) as out_pool,
        ):
            two = const_pool.tile([P, G * F], bf16)
            nc.vector.memset(two[:], 2.0)

            tiles = {}
            slot_insts = {}

            # tiles are always allocated at the max (F) size so the pool
            # layout - and thus SBUF bank alignment - is identical for
            # every iteration; small head tiles just use a prefix
            def loads(i):
                fi = fseq[i]
                a32 = io_pool.tile([P, N_BITS * F], f32, tag="a32")
                b32 = io_pool.tile([P, N_BITS * F], f32, tag="b32")
                c = cin_pool.tile([P, F], f32, tag="c")
                nc.sync.dma_start(out=a32[:, : N_BITS * fi], in_=bits_ap(A_f, i))
                nc.sync.dma_start(out=b32[:, : N_BITS * fi], in_=bits_ap(B_f, i))
                nc.sync.dma_start(out=c[:, :fi], in_=col_ap(C_f, i))
                tiles[i] = (a32, b32, c)

            def converts(i):
                fi = fseq[i]
                a32, b32, c = tiles[i]
                abf = work_pool.tile([P, N_BITS * F], bf16, tag="abf")
                bbf = work_pool.tile([P, N_BITS * F], bf16, tag="bbf")
                t = work_pool.tile([P, G * F], bf16, tag="t")
                nb = N_BITS * fi
                nc.scalar.copy(out=abf[:, :nb], in_=a32[:, :nb])
                nc.scalar.copy(out=bbf[:, :nb], in_=b32[:, :nb])
                # slots: 2*Cin (chain reset)
                t3 = t[:, : G * fi].rearrange("p (f n) -> p f n", n=G)
                slot_inst = nc.scalar.activation(
                    out=t3[:, :, 8:10],
                    in_=c[:, :fi].unsqueeze(2).broadcast_to([P, fi, 2]),
                    func=Copy,
                    scale=2.0,
                )
                slot_insts[i] = slot_inst.ins
                tiles[i] = (abf, bbf, t)

            def compute(i):
                fi = fseq[i]
                Wi = G * fi
                abf, bbf, t = tiles.pop(i)
                sc = work_pool.tile([P, G * F + 1], bf16, tag="sc")
                s32 = out_pool.tile([P, N_BITS * F], f32, tag="s32")
                k = out_pool.tile([P, F], f32, tag="k")

                t3 = t[:, :Wi].rearrange("p (f n) -> p f n", n=G)
                a3 = abf[:, : N_BITS * fi].rearrange("p (f n) -> p f n", n=N_BITS)
                b3 = bbf[:, : N_BITS * fi].rearrange("p (f n) -> p f n", n=N_BITS)

                # bits: t = a + b in {0,1,2}
                nc.vector.tensor_tensor(t3[:, :, 0:8], a3, b3, AluOpType.add)

                # reversed scan, output shifted by one: sc[q+1] = state after q
                # state = (t[q] + state) >= 2 -> every carry, LSB->MSB per row
                nc.vector.tensor_tensor_scan(
                    sc[:, 1 : Wi + 1][:, ::-1],
                    t[:, :Wi][:, ::-1],
                    two[:, 0:Wi][:, ::-1],
                    0.0,
                    AluOpType.add,
                    AluOpType.is_ge,
                )

                # s = (t == 1) XOR carry_in, written as f32 in one fused op
                sc3 = sc[:, 0:Wi].rearrange("p (f n) -> p f n", n=G)
                nc.vector.scalar_tensor_tensor(
                    out=s32[:, : N_BITS * fi].rearrange(
                        "p (f n) -> p f n", n=N_BITS
                    ),
                    in0=t3[:, :, 0:8],
                    scalar=1.0,
                    in1=sc3[:, :, 2:10],
                    op0=AluOpType.is_equal,
                    op1=AluOpType.not_equal,
                )

                # carry-out of row f = state after its MSB = sc[10f+1]
                sc_k = sc[:, 1 : Wi + 1].rearrange("p (f n) -> p f n", n=G)
                k_inst = nc.scalar.copy(
                    out=k[:, :fi].unsqueeze(2), in_=sc_k[:, :, 0:1]
                )
                # keep this (scan-gated) copy BEHIND the next tile's upstream
                # ACT work in the engine FIFO, or it stalls those converts
                if i + 1 in slot_insts:
                    add_dep_helper(
                        k_inst.ins,
                        slot_insts[i + 1],
                        sync=False,
                        reason="ACT upstream-before-downstream",
                    )

                nc.sync.dma_start(out=bits_ap(O_f, i), in_=s32[:, : N_BITS * fi])
                nc.sync.dma_start(out=col_ap(K_f, i), in_=k[:, :fi])

            # software pipeline: loads run 3 ahead, converts 1 ahead, so the
            # ACT FIFO only carries upstream work and the DVE never waits on
            # a cross-engine loop dependency
            loads(0)
            if T > 1:
                loads(1)
            if T > 2:
                loads(2)
            converts(0)
            for i in range(T):
                if i + 3 < T:
                    loads(i + 3)
                if i + 1 < T:
                    converts(i + 1)
                compute(i)

    nc.compile()
    return nc


def _run(nc, in_maps, trace=False):
    from concourse.bass_utils import run_bass_kernel_spmd

    return run_bass_kernel_spmd(
        nc, in_maps, core_ids=list(range(N_CORES)), trace=trace
    )


_NC_CACHE = {}


def kernel(A: np.ndarray, B: np.ndarray, Cin: np.ndarray):
    N = A.shape[0]
    R = N // N_CORES
    A = np.ascontiguousarray(A, dtype=np.float32)
    B = np.ascontiguousarray(B, dtype=np.float32)
    Cin = np.ascontiguousarray(Cin, dtype=np.float32)

    if R not in _NC_CACHE:
        _NC_CACHE[R] = build_adder_nc(R, F=256)
    nc = _NC_CACHE[R]
    in_maps = [
        {
            "A": A[i * R : (i + 1) * R],
            "B": B[i * R : (i + 1) * R],
            "Cin": Cin[i * R : (i + 1) * R],
        }
        for i in range(N_CORES)
    ]
    res = _run(nc, in_maps)
    out = np.concatenate([res.results[i]["out"] for i in range(N_CORES)], axis=0)
    cout = np.concatenate([res.results[i]["cout"] for i in range(N_CORES)], axis=0)
    return out, cout


# revision 50
# speedup vs baseline: 1.0086x; 1.0086x over previous
"""8-bit ripple-carry adder on 8 TRN2 NeuronCores.

Full inputs A[N,8], B[N,8] (MSB-first bits in {0,1} as f32), Cin[N,1].
Returns (out[N,8], carry[N,1]) matching the reference.

Strategy: pure data-parallel over the batch dim (N/8 rows per core).
Per core, rows are tiled [128 partitions x F rows]. Per row the carry
recurrence is c' = (a + b + c >= 2); with t = a + b laid out in 10-wide
groups [b7..b0, slot, slot] (slot = 2*Cin, which forces the next state to
Cin and so resets the chain between rows), a single reversed DVE
tensor_tensor_scan  state = (t + state) >= 2  computes every carry of
every row. Sum bits are (t == 1) XOR carry_in.

bf16 is used for all DVE elementwise ops (values stay in {0,1,2} -
exact), keeping them in the DVE's 2x/4x perf modes; f32<->bf16 converts
run on the Scalar engine, which has its own SBUF ports. GPSIMD is
deliberately unused (it shares SBUF ports with the DVE; concurrent
GPSIMD ops slow DVE tensor ops ~3x, measured). Deep tile-pool buffering
on the input side keeps the DMA stream running ahead of compute.
"""

import sys

import numpy as np

if "/opt/trn_rl_repo" not in sys.path:
    sys.path.insert(0, "/opt/trn_rl_repo")

N_BITS = 8
P = 128
N_CORES = 8


def build_adder_nc(R: int, F: int):
    """Per-core Bass program for an R-row shard, F rows/partition/tile."""
    import concourse.bacc as bacc
    import concourse.mybir as mybir
    from concourse.mybir import AluOpType
    from concourse.tile import TileContext, add_dep_helper

    f32 = mybir.dt.float32
    bf16 = mybir.dt.bfloat16
    Copy = mybir.ActivationFunctionType.Copy
    G = 10  # group width: 8 bits MSB-first + 2 reset slots

    # uniform tiles of F rows/partition (non-uniform head/tail tiles were
    # measured slower - the schedule is sensitive to the steady rhythm)
    rpp = R // P
    assert R % P == 0 and rpp % F == 0
    fseq = [F] * (rpp // F)
    T = len(fseq)
    row0s = []
    r0 = 0
    for fi in fseq:
        row0s.append(r0)
        r0 += P * fi

    nc = bacc.Bacc("TRN2", target_bir_lowering=False, debug=False)

    A = nc.declare_dram_parameter("A", [R, N_BITS], f32, isOutput=False)
    B = nc.declare_dram_parameter("B", [R, N_BITS], f32, isOutput=False)
    CIN = nc.declare_dram_parameter("Cin", [R, 1], f32, isOutput=False)
    OUT = nc.declare_dram_parameter("out", [R, N_BITS], f32, isOutput=True)
    COUT = nc.declare_dram_parameter("cout", [R, 1], f32, isOutput=True)

    A_f = A[:].flatten()
    B_f = B[:].flatten()
    C_f = CIN[:].flatten()
    O_f = OUT[:].flatten()
    K_f = COUT[:].flatten()

    def bits_ap(flat, i):
        r0, fi = row0s[i], fseq[i]
        return flat[r0 * N_BITS : (r0 + P * fi) * N_BITS].rearrange(
            "(p x) -> p x", p=P
        )

    def col_ap(flat, i):
        r0, fi = row0s[i], fseq[i]
        return flat[r0 : r0 + P * fi].rearrange("(p x) -> p x", p=P)

    with TileContext(nc) as tc:
        with (
            tc.tile_pool(name="const", bufs=1) as const_pool,
            tc.tile_pool(name="io", bufs=4) as io_pool,
            tc.tile_pool(name="cin", bufs=6) as cin_pool,
            tc.tile_pool(name="work", bufs=3) as work_pool,
            tc.tile_pool(name="outp", bufs=3) as out_pool,
        ):
            two = const_pool.tile([P, G * F], bf16)
            nc.vector.memset(two[:], 2.0)

            tiles = {}
            slot_insts = {}

            # tiles are always allocated at the max (F) size so the pool
            # layout - and thus SBUF bank alignment - is identical for
            # every iteration. Tile 0 is processed in two half-ranges over
            # the same buffers so the first scan starts sooner: combined
            # with the sub-DMA-pace DVE cycle this lets the DVE catch the
            # DMA stream before the final tile.
            def parts(i):
                if i == 0:
                    return [(0, F // 2), (F // 2, F)]
                return [(0, fseq[i])]

            def loads(i):
                a32 = io_pool.tile([P, N_BITS * F], f32, tag="a32")
                b32 = io_pool.tile([P, N_BITS * F], f32, tag="b32")
                c = cin_pool.tile([P, F], f32, tag="c")
                for f0, f1 in parts(i):
                    b0, b1 = f0 * N_BITS, f1 * N_BITS
                    nc.sync.dma_start(
                        out=a32[:, b0:b1], in_=bits_ap(A_f, i)[:, b0:b1]
                    )
                    nc.sync.dma_start(
                        out=b32[:, b0:b1], in_=bits_ap(B_f, i)[:, b0:b1]
                    )
                    nc.sync.dma_start(out=c[:, f0:f1], in_=col_ap(C_f, i)[:, f0:f1])
                tiles[i] = (a32, b32, c)

            def converts(i):
                a32, b32, c = tiles[i]
                abf = work_pool.tile([P, N_BITS * F], bf16, tag="abf")
                bbf = work_pool.tile([P, N_BITS * F], bf16, tag="bbf")
                t = work_pool.tile([P, G * F], bf16, tag="t")
                t3 = t[:].rearrange("p (f n) -> p f n", n=G)
                for f0, f1 in parts(i):
                    b0, b1 = f0 * N_BITS, f1 * N_BITS
                    nc.scalar.copy(out=abf[:, b0:b1], in_=a32[:, b0:b1])
                    nc.scalar.copy(out=bbf[:, b0:b1], in_=b32[:, b0:b1])
                    # slots: 2*Cin (chain reset)
                    slot_inst = nc.scalar.activation(
                        out=t3[:, f0:f1, 8:10],
                        in_=c[:, f0:f1].unsqueeze(2).broadcast_to(
                            [P, f1 - f0, 2]
                        ),
                        func=Copy,
                        scale=2.0,
                    )
                slot_insts[i] = slot_inst.ins
                tiles[i] = (abf, bbf, t)

            def compute(i):
                fi = fseq[i]
                Wi = G * fi
                abf, bbf, t = tiles.pop(i)
                sc = work_pool.tile([P, G * F + 1], bf16, tag="sc")
                s32 = out_pool.tile([P, N_BITS * F], f32, tag="s32")
                k = out_pool.tile([P, F], f32, tag="k")

                t3 = t[:].rearrange("p (f n) -> p f n", n=G)
                a3 = abf[:].rearrange("p (f n) -> p f n", n=N_BITS)
                b3 = bbf[:].rearrange("p (f n) -> p f n", n=N_BITS)
                sc3 = sc[:, 0 : G * F].rearrange("p (f n) -> p f n", n=G)

                s32_inst = None
                for f0, f1 in parts(i):
                    b0, b1 = f0 * N_BITS, f1 * N_BITS
                    w0, w1 = f0 * G, f1 * G

                    # bits: t = a + b in {0,1,2}
                    nc.vector.tensor_tensor(
                        t3[:, f0:f1, 0:8], a3[:, f0:f1], b3[:, f0:f1],
                        AluOpType.add,
                    )

                    # reversed scan, output shifted by one:
                    # sc[q+1] = state after q;
                    # state = (t[q] + state) >= 2 -> every carry, LSB->MSB
                    nc.vector.tensor_tensor_scan(
                        sc[:, w0 + 1 : w1 + 1][:, ::-1],
                        t[:, w0:w1][:, ::-1],
                        two[:, w0:w1][:, ::-1],
                        0.0,
                        AluOpType.add,
                        AluOpType.is_ge,
                    )

                    # p = (t == 1) at 4x, s = p XOR carry_in at 2x, reusing
                    # the dead abf/bbf buffers (identical pool layout); the
                    # f32 convert of s runs on ACT
                    nc.vector.tensor_scalar(
                        abf[:, b0:b1], t3[:, f0:f1, 0:8], 1.0, None,
                        AluOpType.is_equal,
                    )
                    nc.vector.tensor_tensor(
                        bbf[:, b0:b1].rearrange("p (f n) -> p f n", n=N_BITS),
                        abf[:, b0:b1].rearrange("p (f n) -> p f n", n=N_BITS),
                        sc3[:, f0:f1, 2:10],
                        AluOpType.not_equal,
                    )
                    s32_inst = nc.scalar.copy(
                        out=s32[:, b0:b1], in_=bbf[:, b0:b1]
                    )
                    nc.sync.dma_start(
                        out=bits_ap(O_f, i)[:, b0:b1], in_=s32[:, b0:b1]
                    )

                # carry-out of row f = state after its MSB = sc[10f+1]
                sc_k = sc[:, 1 : Wi + 1].rearrange("p (f n) -> p f n", n=G)
                k_inst = nc.scalar.copy(
                    out=k[:, :fi].unsqueeze(2), in_=sc_k[:, :, 0:1]
                )
                # keep these (scan/s-gated) copies BEHIND the next tile's
                # upstream ACT work in the engine FIFO, or they stall it
                if i + 1 in slot_insts:
                    for inst in (k_inst, s32_inst):
                        add_dep_helper(
                            inst.ins,
                            slot_insts[i + 1],
                            sync=False,
                            reason="ACT upstream-before-downstream",
                        )

                nc.sync.dma_start(out=col_ap(K_f, i), in_=k[:, :fi])

            # software pipeline: loads run 3 ahead, converts 1 ahead, so the
            # ACT FIFO only carries upstream work and the DVE never waits on
            # a cross-engine loop dependency
            loads(0)
            if T > 1:
                loads(1)
            if T > 2:
                loads(2)
            converts(0)
            for i in range(T):
                if i + 3 < T:
                    loads(i + 3)
                if i + 1 < T:
                    converts(i + 1)
                compute(i)

    nc.compile()
    return nc


def _run(nc, in_maps, trace=False):
    from concourse.bass_utils import run_bass_kernel_spmd

    return run_bass_kernel_spmd(
        nc, in_maps, core_ids=list(range(N_CORES)), trace=trace
    )


_NC_CACHE = {}


def kernel(A: np.ndarray, B: np.ndarray, Cin: np.ndarray):
    N = A.shape[0]
    R = N // N_CORES
    A = np.ascontiguousarray(A, dtype=np.float32)
    B = np.ascontiguousarray(B, dtype=np.float32)
    Cin = np.ascontiguousarray(Cin, dtype=np.float32)

    if R not in _NC_CACHE:
        _NC_CACHE[R] = build_adder_nc(R, F=256)
    nc = _NC_CACHE[R]
    in_maps = [
        {
            "A": A[i * R : (i + 1) * R],
            "B": B[i * R : (i + 1) * R],
            "Cin": Cin[i * R : (i + 1) * R],
        }
        for i in range(N_CORES)
    ]
    res = _run(nc, in_maps)
    out = np.concatenate([res.results[i]["out"] for i in range(N_CORES)], axis=0)
    cout = np.concatenate([res.results[i]["cout"] for i in range(N_CORES)], axis=0)
    return out, cout


# revision 51
# speedup vs baseline: 1.1206x; 1.1110x over previous
"""8-bit ripple-carry adder on 8 TRN2 NeuronCores.

Full inputs A[N,8], B[N,8] (MSB-first bits in {0,1} as f32), Cin[N,1].
Returns (out[N,8], carry[N,1]) matching the reference.

Strategy: pure data-parallel over the batch dim (N/8 rows per core).
Per core, rows are tiled [128 partitions x F rows]. Per row the carry
recurrence is c' = (a + b + c >= 2); with t = a + b laid out in 10-wide
groups [b7..b0, slot, slot] (slot = 2*Cin, which forces the next state to
Cin and so resets the chain between rows), a single reversed DVE
tensor_tensor_scan  state = (t + state) >= 2  computes every carry of
every row. Sum bits are (t == 1) XOR carry_in.

bf16 is used for all DVE elementwise ops (values stay in {0,1,2} -
exact), keeping them in the DVE's 2x/4x perf modes; f32<->bf16 converts
run on the Scalar engine, which has its own SBUF ports. GPSIMD is
deliberately unused (it shares SBUF ports with the DVE; concurrent
GPSIMD ops slow DVE tensor ops ~3x, measured). Deep tile-pool buffering
on the input side keeps the DMA stream running ahead of compute.
"""

import sys

import numpy as np

if "/opt/trn_rl_repo" not in sys.path:
    sys.path.insert(0, "/opt/trn_rl_repo")

N_BITS = 8
P = 128
N_CORES = 8


def build_adder_nc(R: int, F: int):
    """Per-core Bass program for an R-row shard, F rows/partition/tile."""
    import concourse.bacc as bacc
    import concourse.mybir as mybir
    from concourse.mybir import AluOpType
    from concourse.tile import TileContext, add_dep_helper

    f32 = mybir.dt.float32
    bf16 = mybir.dt.bfloat16
    Copy = mybir.ActivationFunctionType.Copy
    G = 10  # group width: 8 bits MSB-first + 2 reset slots

    # uniform tiles of F rows/partition (non-uniform head/tail tiles were
    # measured slower - the schedule is sensitive to the steady rhythm)
    rpp = R // P
    assert R % P == 0 and rpp % F == 0
    fseq = [F] * (rpp // F)
    T = len(fseq)
    row0s = []
    r0 = 0
    for fi in fseq:
        row0s.append(r0)
        r0 += P * fi

    nc = bacc.Bacc("TRN2", target_bir_lowering=False, debug=False)

    A = nc.declare_dram_parameter("A", [R, N_BITS], f32, isOutput=False)
    B = nc.declare_dram_parameter("B", [R, N_BITS], f32, isOutput=False)
    CIN = nc.declare_dram_parameter("Cin", [R, 1], f32, isOutput=False)
    OUT = nc.declare_dram_parameter("out", [R, N_BITS], f32, isOutput=True)
    COUT = nc.declare_dram_parameter("cout", [R, 1], f32, isOutput=True)

    A_f = A[:].flatten()
    B_f = B[:].flatten()
    C_f = CIN[:].flatten()
    O_f = OUT[:].flatten()
    K_f = COUT[:].flatten()

    def bits_ap(flat, i):
        r0, fi = row0s[i], fseq[i]
        return flat[r0 * N_BITS : (r0 + P * fi) * N_BITS].rearrange(
            "(p x) -> p x", p=P
        )

    def col_ap(flat, i):
        r0, fi = row0s[i], fseq[i]
        return flat[r0 : r0 + P * fi].rearrange("(p x) -> p x", p=P)

    with TileContext(nc) as tc:
        with (
            tc.tile_pool(name="const", bufs=1) as const_pool,
            tc.tile_pool(name="io", bufs=4) as io_pool,
            tc.tile_pool(name="cin", bufs=6) as cin_pool,
            tc.tile_pool(name="work", bufs=3) as work_pool,
            tc.tile_pool(name="outp", bufs=3) as out_pool,
        ):
            two = const_pool.tile([P, G * F], bf16)
            nc.vector.memset(two[:], 2.0)

            tiles = {}
            slot_insts = {}

            # tiles are always allocated at the max (F) size so the pool
            # layout - and thus SBUF bank alignment - is identical for
            # every iteration; small head tiles just use a prefix
            def loads(i):
                fi = fseq[i]
                a32 = io_pool.tile([P, N_BITS * F], f32, tag="a32")
                b32 = io_pool.tile([P, N_BITS * F], f32, tag="b32")
                c = cin_pool.tile([P, F], f32, tag="c")
                nc.sync.dma_start(out=a32[:, : N_BITS * fi], in_=bits_ap(A_f, i))
                nc.sync.dma_start(out=b32[:, : N_BITS * fi], in_=bits_ap(B_f, i))
                nc.sync.dma_start(out=c[:, :fi], in_=col_ap(C_f, i))
                tiles[i] = (a32, b32, c)

            def converts(i):
                fi = fseq[i]
                a32, b32, c = tiles[i]
                abf = work_pool.tile([P, N_BITS * F], bf16, tag="abf")
                bbf = work_pool.tile([P, N_BITS * F], bf16, tag="bbf")
                t = work_pool.tile([P, G * F], bf16, tag="t")
                nb = N_BITS * fi
                nc.scalar.copy(out=abf[:, :nb], in_=a32[:, :nb])
                nc.scalar.copy(out=bbf[:, :nb], in_=b32[:, :nb])
                # slots: 2*Cin (chain reset)
                t3 = t[:, : G * fi].rearrange("p (f n) -> p f n", n=G)
                slot_inst = nc.scalar.activation(
                    out=t3[:, :, 8:10],
                    in_=c[:, :fi].unsqueeze(2).broadcast_to([P, fi, 2]),
                    func=Copy,
                    scale=2.0,
                )
                slot_insts[i] = slot_inst.ins
                tiles[i] = (abf, bbf, t)

            def compute(i):
                fi = fseq[i]
                Wi = G * fi
                abf, bbf, t = tiles.pop(i)
                sc = work_pool.tile([P, G * F + 1], bf16, tag="sc")
                s32 = out_pool.tile([P, N_BITS * F], f32, tag="s32")
                k = out_pool.tile([P, F], f32, tag="k")

                t3 = t[:, :Wi].rearrange("p (f n) -> p f n", n=G)
                a3 = abf[:, : N_BITS * fi].rearrange("p (f n) -> p f n", n=N_BITS)
                b3 = bbf[:, : N_BITS * fi].rearrange("p (f n) -> p f n", n=N_BITS)

                # bits: t = a + b in {0,1,2}
                nc.vector.tensor_tensor(t3[:, :, 0:8], a3, b3, AluOpType.add)

                # reversed scan, output shifted by one: sc[q+1] = state after q
                # state = (t[q] + state) >= 2 -> every carry, LSB->MSB per row
                nc.vector.tensor_tensor_scan(
                    sc[:, 1 : Wi + 1][:, ::-1],
                    t[:, :Wi][:, ::-1],
                    two[:, 0:Wi][:, ::-1],
                    0.0,
                    AluOpType.add,
                    AluOpType.is_ge,
                )

                # p = (t == 1) at 4x, s = p XOR carry_in at 2x, reusing the
                # dead abf/bbf buffers (identical pool layout); the f32
                # convert of s runs on ACT
                nb = N_BITS * fi
                sc3 = sc[:, 0:Wi].rearrange("p (f n) -> p f n", n=G)
                nc.vector.tensor_scalar(
                    abf[:, :nb], t3[:, :, 0:8], 1.0, None, AluOpType.is_equal
                )
                nc.vector.tensor_tensor(
                    bbf[:, :nb].rearrange("p (f n) -> p f n", n=N_BITS),
                    abf[:, :nb].rearrange("p (f n) -> p f n", n=N_BITS),
                    sc3[:, :, 2:10],
                    AluOpType.not_equal,
                )
                s32_inst = nc.scalar.copy(out=s32[:, :nb], in_=bbf[:, :nb])

                # carry-out of row f = state after its MSB = sc[10f+1]
                sc_k = sc[:, 1 : Wi + 1].rearrange("p (f n) -> p f n", n=G)
                k_inst = nc.scalar.copy(
                    out=k[:, :fi].unsqueeze(2), in_=sc_k[:, :, 0:1]
                )
                # keep these (scan/s-gated) copies BEHIND the next tile's
                # upstream ACT work in the engine FIFO, or they stall it
                if i + 1 in slot_insts:
                    for inst in (k_inst, s32_inst):
                        add_dep_helper(
                            inst.ins,
                            slot_insts[i + 1],
                            sync=False,
                            reason="ACT upstream-before-downstream",
                        )

                nc.sync.dma_start(out=bits_ap(O_f, i), in_=s32[:, : N_BITS * fi])
                nc.sync.dma_start(out=col_ap(K_f, i), in_=k[:, :fi])

            # software pipeline: loads run 3 ahead, converts 1 ahead, so the
            # ACT FIFO only carries upstream work and the DVE never waits on
            # a cross-engine loop dependency
            loads(0)
            if T > 1:
                loads(1)
            if T > 2:
                loads(2)
            converts(0)
            for i in range(T):
                if i + 3 < T:
                    loads(i + 3)
                if i + 1 < T:
                    converts(i + 1)
                compute(i)

    nc.compile()
    return nc


def _run(nc, in_maps, trace=False):
    from concourse.bass_utils import run_bass_kernel_spmd

    return run_bass_kernel_spmd(
        nc, in_maps, core_ids=list(range(N_CORES)), trace=trace
    )


_NC_CACHE = {}


def kernel(A: np.ndarray, B: np.ndarray, Cin: np.ndarray):
    N = A.shape[0]
    R = N // N_CORES
    A = np.ascontiguousarray(A, dtype=np.float32)
    B = np.ascontiguousarray(B, dtype=np.float32)
    Cin = np.ascontiguousarray(Cin, dtype=np.float32)

    if R not in _NC_CACHE:
        _NC_CACHE[R] = build_adder_nc(R, F=256)
    nc = _NC_CACHE[R]
    in_maps = [
        {
            "A": A[i * R : (i + 1) * R],
            "B": B[i * R : (i + 1) * R],
            "Cin": Cin[i * R : (i + 1) * R],
        }
        for i in range(N_CORES)
    ]
    res = _run(nc, in_maps)
    out = np.concatenate([res.results[i]["out"] for i in range(N_CORES)], axis=0)
    cout = np.concatenate([res.results[i]["cout"] for i in range(N_CORES)], axis=0)
    return out, cout
